# revision 2
# baseline (speedup 1.0000x reference)
"""Causal MHA with RoPE on 8 TRN2 NeuronCores — v2 (all-bf16 matmuls).

Sharding: data-parallel over batch (2) x tensor-parallel over heads (4 groups
of 4 heads) = 8 cores. Core c handles batch c//4, head group c%4.

v2 design vs baseline:
  - all matmul operands bf16 (fp32 PSUM accumulate); fp32r only where free
  - PV in A[q, f] orientation: out A[q=128, 4h, 65] (ones column in vt gives
    row sums at col 64) -> half the PE rows of the attnT orientation
  - normalization fused: reciprocal(A[:,:,64]) then one broadcast multiply
    writing bf16 attn; O-projection needs attnT so two PE transposes per qb
    (identity matmul) land in the tail bytes of the A psum bank (bitcast)
  - no psr/rbc broadcast matmuls, no sra/srb copies
  - q-tiles of 256 for scores/exp so PSUM fits: mm(2 banks) ss(3) A(2)
  - kb-major PV: probs tiles live ~1 kb step -> small SBUF pool
  - O output bf16, host sums partials in fp32
"""
import sys
import os

for _p in ("/opt/trn_rl_repo", "/root/.axon_site/_ro/trn_rl_repo"):
    if os.path.isdir(_p) and _p not in sys.path:
        sys.path.insert(0, _p)

import numpy as np
import ml_dtypes

import concourse.mybir as mybir
import concourse.tile as tile
from concourse import bacc
from concourse.bass_utils import run_bass_kernel_spmd

F32 = mybir.dt.float32
BF16 = mybir.dt.bfloat16
AF = mybir.ActivationFunctionType
MULT = mybir.AluOpType.mult
ADD = mybir.AluOpType.add
NPBF = ml_dtypes.bfloat16

B, S, D = 2, 2048, 1024
H, DK = 16, 64
THETA = 10000.0
NCORES = 8
GROUPS = 4          # head groups (tensor parallel)
GH = H // GROUPS    # heads per group = 4
GF = GH * DK        # features per group = 256
SWAP_MASK = [i ^ 1 for i in range(32)]
KVER = 25  # bump on any kernel change: busts the HLO-shape-keyed NEFF cache

_CACHED = {}
KCUT = int(os.environ.get("KCUT", "5"))  # bisect: 1=proj 2=+scores 3=+pv 4=+finalize 5=full

QT = 256            # q tile for scores/exp
NQT = S // QT       # 8
KB = 128            # k block
NKB = S // KB       # 16


def _build_nc(iters=1):
    nc = bacc.Bacc("TRN2", target_bir_lowering=False, debug=False, num_devices=NCORES)
    xT = nc.dram_tensor("xT", [D, S], BF16, kind="ExternalInput").ap()
    wqkT = nc.dram_tensor("wqkT", [D, 2 * GF], BF16, kind="ExternalInput").ap()
    wvT = nc.dram_tensor("wvT", [D, GF], BF16, kind="ExternalInput").ap()
    woT = nc.dram_tensor("woT", [GF, D], BF16, kind="ExternalInput").ap()
    cosf = nc.dram_tensor("cosf", [128, S], BF16, kind="ExternalInput").ap()
    sins = nc.dram_tensor("sins", [128, S], BF16, kind="ExternalInput").ap()
    tri = nc.dram_tensor("tri", [128, 128], BF16, kind="ExternalInput").ap()
    iden = nc.dram_tensor("iden", [128, 128], BF16, kind="ExternalInput").ap()
    nc.dram_tensor("cachebust", [iters, KVER], F32, kind="ExternalInput")
    out = nc.dram_tensor("out", [S, D], BF16, kind="ExternalOutput").ap()

    with tile.TileContext(nc) as tc:
        with tc.tile_pool(name="const", bufs=1) as cpool, \
             tc.tile_pool(name="xt", bufs=8) as xpool, \
             tc.tile_pool(name="big", bufs=1) as bpool, \
             tc.tile_pool(name="rope", bufs=2) as rpool, \
             tc.tile_pool(name="probs", bufs=8) as ppool, \
             tc.tile_pool(name="work", bufs=2) as wpool, \
             tc.tile_pool(name="osb", bufs=4) as opool, \
             tc.tile_pool(name="psum", bufs=1, space="PSUM") as psum:

            # ---- loads, ordered by first use ----
            wqk_sb = cpool.tile([128, 8, 2 * GF], BF16, tag="wqk")
            wv_sb = cpool.tile([128, 8, GF], BF16, tag="wv")
            wo_sb = cpool.tile([128, 2, D], BF16, tag="wo")
            cos_sb = cpool.tile([128, S], BF16, tag="cos")
            sin_sb = cpool.tile([128, S], BF16, tag="sin")
            tri_sb = cpool.tile([128, 128], BF16, tag="tri")
            iden_sb = cpool.tile([128, 128], BF16, tag="iden")
            zrow_sb = cpool.tile([1, 512], BF16, tag="zrow")
            xt_tiles = [xpool.tile([128, S], BF16, tag="xt", name=f"xt{i}")
                        for i in range(8)]

            for dc in range(8):
                nc.sync.dma_start(wqk_sb[:, dc, :], wqkT[dc * 128:(dc + 1) * 128, :])
                nc.sync.dma_start(xt_tiles[dc][:, 0:512],
                                  xT[dc * 128:(dc + 1) * 128, 0:512])
            nc.sync.dma_start(cos_sb[:, 0:512], cosf[:, 0:512])
            nc.sync.dma_start(sin_sb[:, 0:512], sins[:, 0:512])
            nc.sync.dma_start(wv_sb[:], wvT.rearrange("(dc p) n -> p dc n", p=128))
            nc.sync.dma_start(tri_sb[:], tri)
            nc.sync.dma_start(iden_sb[:], iden)
            for dc in range(8):
                nc.sync.dma_start(xt_tiles[dc][:, 512:1024],
                                  xT[dc * 128:(dc + 1) * 128, 512:1024])
            nc.sync.dma_start(cos_sb[:, 512:1024], cosf[:, 512:1024])
            nc.sync.dma_start(sin_sb[:, 512:1024], sins[:, 512:1024])
            for dc in range(8):
                nc.sync.dma_start(xt_tiles[dc][:, 1024:S],
                                  xT[dc * 128:(dc + 1) * 128, 1024:S])
            nc.sync.dma_start(cos_sb[:, 1024:S], cosf[:, 1024:S])
            nc.sync.dma_start(sin_sb[:, 1024:S], sins[:, 1024:S])
            nc.sync.dma_start(wo_sb[:], woT.rearrange("(fc p) n -> p fc n", p=128))

            # warm the exp table (input: first-loaded tile, so this runs early)
            warm = cpool.tile([1, 1], F32, tag="warm")
            nc.scalar.activation(warm[:], wqk_sb[0:1, 0, 0:1], AF.Exp, scale=1.0)
            nc.gpsimd.memset(zrow_sb[:], 0.0)

            for _it in range(iters):
                # [64, 8, S]: head-halves in free slots, all at partition base 0
                # (bf16 matmul operands at partition base 64 hang the device)
                qkT = bpool.tile([64, 8, S], BF16, tag="qkT", name=f"qkT{_it}")
                attnT = bpool.tile([128, 2, S], BF16, tag="attnT", name=f"attnT{_it}")
                vt = bpool.tile([128, NKB, GH, 65], BF16, tag="vt", name=f"vt{_it}")
                # ones column for row sums: contiguous whole-tile memset, V
                # copies then overwrite cols 0:64 (strided memset avoided)
                nc.gpsimd.memset(vt[:].rearrange("p a b c -> p (a b c)"), 1.0)

                # A psum accumulators: 2 live banks (qb parity), tail bytes
                # reused as bf16 transpose target
                def a_tile(qb):
                    return psum.tile([128, 512], F32, tag=f"A{qb % 2}", bufs=1,
                                     name=f"A{qb % 2}")

                A_map = {}
                pending_o = []

                def emit_pv(qt, kb, pr):
                    """PV matmuls of kb into the A accumulators of q tile qt."""
                    q0 = qt * QT
                    qb_lo = max(kb, q0 // KB)
                    for pair in range(2):
                        for qb in range(qb_lo, (q0 + QT) // KB):
                            A = A_map[qb]
                            Av = A[:].rearrange("p (h n) -> p h n", h=4)
                            poff = qb * KB - q0
                            for h2 in range(2):
                                h = pair * 2 + h2
                                last = (kb == qb and pair == 1 and h2 == 1)
                                nc.tensor.matmul(
                                    Av[:, h, 0:65],
                                    pr[:, pair, h2, poff:poff + 128],
                                    vt[:, kb, h, :],
                                    start=False, stop=last,
                                    skip_group_check=not last)

                def attention_step(qt, pending):
                    """scores/exp for q tile qt; PV pipelined one kb behind.
                    pending: list of deferred emitters (flushed after kb=0
                    scores to hide their latency)."""
                    q0 = qt * QT
                    nkb = (q0 + QT) // KB  # kb in [0, nkb)
                    inflight = []          # [(kb, prs)] awaiting PV, depth 2
                    for kb in range(nkb):
                        diag = kb * KB >= q0  # kb inside this q tile's diagonal
                        lam = max(kb * KB - q0, 0)
                        ss = psum.tile([128, 2, 2, QT], F32, tag="ss", bufs=2)
                        qsl = slice(q0 + lam, q0 + QT)
                        ksl = slice(kb * KB, (kb + 1) * KB)
                        for pair in range(2):
                            for half in range(2):
                                qs, ks = 2 * pair + half, 4 + 2 * pair + half
                                nc.tensor.matmul(ss[:, pair, half, lam:QT],
                                                 qkT[:, ks, ksl],
                                                 qkT[:, qs, qsl],
                                                 start=True, stop=True)
                        pr = ppool.tile([128, 2, 2, QT], BF16, tag="pr")
                        if int(os.environ.get("KSUB", "2")) < 1:
                            inflight.append((kb, pr))
                            if len(inflight) > 2:
                                inflight.pop(0)
                            continue
                        if lam == 0:
                            # contiguous: single exp over the whole tile
                            nc.scalar.activation(
                                pr[:].rearrange("p a b n -> p (a b n)"),
                                ss[:].rearrange("p a b n -> p (a b n)"),
                                AF.Exp, scale=0.125)
                        else:
                            # diagonal: per-pair exp keeps the AP at 2 free dims
                            for pair in range(2):
                                nc.scalar.activation(pr[:, pair, :, lam:QT],
                                                     ss[:, pair, :, lam:QT],
                                                     AF.Exp, scale=0.125)
                        if diag and int(os.environ.get("KSUB", "2")) >= 2:
                            dsl = slice(lam, lam + 128)
                            for pair in range(2):
                                nc.vector.tensor_tensor(
                                    pr[:, pair, :, dsl], pr[:, pair, :, dsl],
                                    tri_sb[:, None, :].to_broadcast([128, 2, 128]),
                                    MULT)

                        inflight.append((kb, pr))
                        if len(inflight) > 2:
                            pkb, ppr = inflight.pop(0)
                            if KCUT >= 3:
                                emit_pv(qt, pkb, ppr)
                    for pkb, ppr in inflight:
                        if KCUT >= 3:
                            emit_pv(qt, pkb, ppr)
                    for qb in range(q0 // KB, (q0 + QT) // KB):
                        if KCUT >= 4:
                            pending.append(finalize_qb(qb))
                        else:
                            A_map.pop(qb)

                def finalize_qb(qb):
                    """normalize + transpose into attnT; returns deferred O-proj."""
                    A = A_map.pop(qb)
                    Av = A[:].rearrange("p (h n) -> p h n", h=4)
                    rcp = wpool.tile([128, 4], F32, tag="rcp")
                    nc.vector.reciprocal(rcp[:], Av[:, :, 64])
                    asb = wpool.tile([128, 4, 64], BF16, tag="asb")
                    nc.vector.tensor_tensor(
                        asb[:], Av[:, :, 0:64],
                        rcp[:, :, None].to_broadcast([128, 4, 64]), MULT)
                    qsl = slice(qb * KB, (qb + 1) * KB)
                    for ft in range(2):
                        # transpose lands in the tail bytes of the drained A bank
                        pT = A[:, 384 + 64 * ft:448 + 64 * ft].bitcast(BF16)
                        nc.tensor.matmul(pT, asb[:, 2 * ft:2 * ft + 2, :]
                                         .rearrange("p a b -> p (a b)"),
                                         iden_sb[:], is_transpose=True,
                                         start=True, stop=True,
                                         skip_group_check=True)
                        nc.vector.tensor_copy(attnT[:, ft, qsl], pT)

                    def o_proj():
                        for nh in range(2):
                            nsl = slice(nh * 512, (nh + 1) * 512)
                            pso = psum.tile([128, 512], F32, tag="mm", bufs=2)
                            nc.tensor.matmul(pso[:], attnT[:, 0, qsl], wo_sb[:, 0, nsl],
                                             start=True, stop=False)
                            nc.tensor.matmul(pso[:], attnT[:, 1, qsl], wo_sb[:, 1, nsl],
                                             start=False, stop=True)
                            ob = opool.tile([128, 512], BF16, tag="osb")
                            nc.vector.tensor_copy(ob[:], pso[:])
                            nc.sync.dma_start(out[qsl, nsl], ob[:])
                    return o_proj

                # ---------------- main pipeline over t (s-tiles of 512) -------
                def v_proj(t, sbp):
                    kb0 = 4 * t + 2 * sbp
                    psv = psum.tile([128, 512], F32, tag="mm", bufs=2)
                    pv_v = psv[:].rearrange("p (b n) -> p b n", b=2)
                    for par in range(2):
                        sb = kb0 + par
                        for dc in range(8):
                            nc.tensor.matmul(
                                pv_v[:, par, :],
                                xt_tiles[dc][:, sb * 128:(sb + 1) * 128],
                                wv_sb[:, dc, :], start=(dc == 0), stop=(dc == 7))
                    nc.vector.tensor_copy(
                        vt[:, kb0:kb0 + 2, :, 0:64],
                        pv_v[:].rearrange("p b (h d) -> p b h d", h=4))

                for t in range(4):
                    tsl = slice(t * 512, (t + 1) * 512)
                    # QK projection + rope, V projection interleaved
                    for c in (0, 2, 1, 3):
                        if t == 0 and c < 2:
                            ps = psum.tile([128, 512], F32, tag=f"A{c}", bufs=1,
                                           name=f"psA{c}")
                        else:
                            ps = psum.tile([128, 512], F32, tag="mm", bufs=2)
                        for dc in range(8):
                            nc.tensor.matmul(
                                ps[:], wqk_sb[:, dc, c * 128:(c + 1) * 128],
                                xt_tiles[dc][:, tsl],
                                start=(dc == 0), stop=(dc == 7))
                        sh = rpool.tile([128, 512], F32, tag="sh")
                        nc.vector.stream_shuffle(sh[:], ps[:], SWAP_MASK)
                        m1 = rpool.tile([128, 512], BF16, tag="m1")
                        nc.vector.tensor_tensor(m1[:], ps[:], cos_sb[:, tsl], MULT)
                        m2 = rpool.tile([128, 512], BF16, tag="m2")
                        nc.gpsimd.tensor_tensor(m2[:], sh[:], sin_sb[:, tsl], MULT)
                        # c -> slots: Q pairs at 2c, K pairs at 2c (c>=2 maps +4)
                        nc.gpsimd.tensor_tensor(qkT[:, 2 * c, tsl],
                                                m1[0:64, :], m2[0:64, :], ADD)
                        nc.vector.tensor_tensor(qkT[:, 2 * c + 1, tsl],
                                                m1[64:128, :], m2[64:128, :], ADD)
                        if c in (1, 3):
                            v_proj(t, (c - 1) // 2)
                    # attention for the two q tiles covered by this t
                    for qhalf in range(2):
                        if KCUT < 2:
                            continue
                        qt = 2 * t + qhalf
                        for qb in range(2 * qt, 2 * qt + 2):
                            A_map[qb] = a_tile(qb)
                            # explicit zeroing matmul: real HW has no
                            # region-level zero-on-start; every PV matmul
                            # accumulates (start=False) onto this
                            if KCUT >= 3:
                                nc.tensor.matmul(A_map[qb][:], zrow_sb[0:1, 0:128],
                                                 zrow_sb[0:1, :], start=True,
                                                 stop=False)
                        attention_step(qt, pending_o)
                for fn in pending_o:
                    if KCUT >= 5:
                        fn()
                pending_o.clear()

    nc.compile()
    return nc


def _host_tables(token_positions):
    pos = np.asarray(token_positions, dtype=np.float32)  # [S]
    freq = THETA ** (-np.arange(0, DK, 2, dtype=np.float32) / DK)  # [32]
    f64 = np.repeat(freq, 2)          # [64] freq per feature index
    ang64 = pos[None, :] * f64[:, None]  # [64, S]
    cos64 = np.cos(ang64)
    sin64 = np.sin(ang64)
    # v2 applies the swap BEFORE the sin multiply (shuffle reads the psum
    # directly), so the folded sign pattern is flipped vs the baseline:
    # row 2i needs -sin, row 2i+1 needs +sin.
    sign = np.where(np.arange(DK) % 2 == 0, -1.0, 1.0).astype(np.float32)
    sins64 = sin64 * sign[:, None]
    cosf = np.concatenate([cos64, cos64], axis=0)   # [128, S]
    sins = np.concatenate([sins64, sins64], axis=0)  # [128, S]
    return cosf.astype(NPBF), sins.astype(NPBF)


def kernel(x, Wq, Wk, Wv, Wo, token_positions):
    x = np.asarray(x, dtype=np.float32)
    Wq = np.asarray(Wq, dtype=np.float32)
    Wk = np.asarray(Wk, dtype=np.float32)
    Wv = np.asarray(Wv, dtype=np.float32)
    Wo = np.asarray(Wo, dtype=np.float32)

    if "nc" not in _CACHED:
        _CACHED["nc"] = _build_nc(iters=int(os.environ.get("BENCH_ITERS", "1")))
    nc = _CACHED["nc"]

    cosf, sins = _host_tables(token_positions)
    tri = np.triu(np.ones((128, 128), dtype=np.float32)).astype(NPBF)
    iden = np.eye(128, dtype=NPBF)

    xT = [np.ascontiguousarray(x[b].T).astype(NPBF) for b in range(B)]
    in_maps = []
    for c in range(NCORES):
        b, g = c // GROUPS, c % GROUPS
        R = slice(g * GF, (g + 1) * GF)
        wqkT = np.ascontiguousarray(
            np.concatenate([Wq[R].T, Wk[R].T], axis=1)).astype(NPBF)  # [D, 512]
        wvT = np.ascontiguousarray(Wv[R].T).astype(NPBF)              # [D, 256]
        woT = np.ascontiguousarray(Wo[:, R].T).astype(NPBF)           # [256, D]
        in_maps.append({
            "xT": xT[b], "wqkT": wqkT, "wvT": wvT, "woT": woT,
            "cosf": cosf, "sins": sins, "tri": tri, "iden": iden,
            "cachebust": np.zeros((int(os.environ.get("BENCH_ITERS", "1")), KVER),
                                  dtype=np.float32),
        })

    try:
        res = run_bass_kernel_spmd(nc, in_maps, core_ids=list(range(NCORES)))
    except Exception:
        import time as _time
        _time.sleep(2.0)
        res = run_bass_kernel_spmd(nc, in_maps, core_ids=list(range(NCORES)))
    _CACHED["last_results"] = res
    outs = [np.asarray(r["out"], dtype=np.float32) for r in res.results]
    full = np.empty((B, S, D), dtype=np.float32)
    for b in range(B):
        full[b] = sum(outs[b * GROUPS + g] for g in range(GROUPS))
    return full


# revision 3
# speedup vs baseline: 1.0179x; 1.0179x over previous
"""Causal MHA with RoPE on 8 TRN2 NeuronCores — v2 (all-bf16 matmuls).

Sharding: data-parallel over batch (2) x tensor-parallel over heads (4 groups
of 4 heads) = 8 cores. Core c handles batch c//4, head group c%4.

v2 design vs baseline:
  - all matmul operands bf16 (fp32 PSUM accumulate); fp32r only where free
  - PV in A[q, f] orientation: out A[q=128, 4h, 65] (ones column in vt gives
    row sums at col 64) -> half the PE rows of the attnT orientation
  - normalization fused: reciprocal(A[:,:,64]) then one broadcast multiply
    writing bf16 attn; O-projection needs attnT so two PE transposes per qb
    (identity matmul) land in the tail bytes of the A psum bank (bitcast)
  - no psr/rbc broadcast matmuls, no sra/srb copies
  - q-tiles of 256 for scores/exp so PSUM fits: mm(2 banks) ss(3) A(2)
  - kb-major PV: probs tiles live ~1 kb step -> small SBUF pool
  - O output bf16, host sums partials in fp32
"""
import sys
import os

for _p in ("/opt/trn_rl_repo", "/root/.axon_site/_ro/trn_rl_repo"):
    if os.path.isdir(_p) and _p not in sys.path:
        sys.path.insert(0, _p)

import numpy as np
import ml_dtypes

import concourse.mybir as mybir
import concourse.tile as tile
from concourse import bacc
from concourse.bass_utils import run_bass_kernel_spmd

F32 = mybir.dt.float32
BF16 = mybir.dt.bfloat16
AF = mybir.ActivationFunctionType
MULT = mybir.AluOpType.mult
ADD = mybir.AluOpType.add
NPBF = ml_dtypes.bfloat16

B, S, D = 2, 2048, 1024
H, DK = 16, 64
THETA = 10000.0
NCORES = 8
GROUPS = 4          # head groups (tensor parallel)
GH = H // GROUPS    # heads per group = 4
GF = GH * DK        # features per group = 256
SWAP_MASK = [i ^ 1 for i in range(32)]
KVER = 25  # bump on any kernel change: busts the HLO-shape-keyed NEFF cache

_CACHED = {}

QT = 256            # q tile for scores/exp
NQT = S // QT       # 8
KB = 128            # k block
NKB = S // KB       # 16


def _build_nc(iters=1):
    nc = bacc.Bacc("TRN2", target_bir_lowering=False, debug=False, num_devices=NCORES)
    xT = nc.dram_tensor("xT", [D, S], BF16, kind="ExternalInput").ap()
    wqkT = nc.dram_tensor("wqkT", [D, 2 * GF], BF16, kind="ExternalInput").ap()
    wvT = nc.dram_tensor("wvT", [D, GF], BF16, kind="ExternalInput").ap()
    woT = nc.dram_tensor("woT", [GF, D], BF16, kind="ExternalInput").ap()
    cosf = nc.dram_tensor("cosf", [128, S], BF16, kind="ExternalInput").ap()
    sins = nc.dram_tensor("sins", [128, S], BF16, kind="ExternalInput").ap()
    tri = nc.dram_tensor("tri", [128, 128], BF16, kind="ExternalInput").ap()
    iden = nc.dram_tensor("iden", [128, 128], BF16, kind="ExternalInput").ap()
    nc.dram_tensor("cachebust", [iters, KVER], F32, kind="ExternalInput")
    out = nc.dram_tensor("out", [S, D], BF16, kind="ExternalOutput").ap()

    with tile.TileContext(nc) as tc:
        with tc.tile_pool(name="const", bufs=1) as cpool, \
             tc.tile_pool(name="xt", bufs=8) as xpool, \
             tc.tile_pool(name="big", bufs=1) as bpool, \
             tc.tile_pool(name="rope", bufs=2) as rpool, \
             tc.tile_pool(name="probs", bufs=8) as ppool, \
             tc.tile_pool(name="work", bufs=2) as wpool, \
             tc.tile_pool(name="osb", bufs=4) as opool, \
             tc.tile_pool(name="psum", bufs=1, space="PSUM") as psum:

            # ---- loads, ordered by first use ----
            wqk_sb = cpool.tile([128, 8, 2 * GF], BF16, tag="wqk")
            wv_sb = cpool.tile([128, 8, GF], BF16, tag="wv")
            wo_sb = cpool.tile([128, 2, D], BF16, tag="wo")
            cos_sb = cpool.tile([128, S], BF16, tag="cos")
            sin_sb = cpool.tile([128, S], BF16, tag="sin")
            tri_sb = cpool.tile([128, 128], BF16, tag="tri")
            iden_sb = cpool.tile([128, 128], BF16, tag="iden")
            zrow_sb = cpool.tile([1, 512], BF16, tag="zrow")
            xt_tiles = [xpool.tile([128, S], BF16, tag="xt", name=f"xt{i}")
                        for i in range(8)]

            for dc in range(8):
                nc.sync.dma_start(wqk_sb[:, dc, :], wqkT[dc * 128:(dc + 1) * 128, :])
                nc.sync.dma_start(xt_tiles[dc][:, 0:512],
                                  xT[dc * 128:(dc + 1) * 128, 0:512])
            nc.sync.dma_start(cos_sb[:, 0:512], cosf[:, 0:512])
            nc.sync.dma_start(sin_sb[:, 0:512], sins[:, 0:512])
            nc.sync.dma_start(wv_sb[:], wvT.rearrange("(dc p) n -> p dc n", p=128))
            nc.sync.dma_start(tri_sb[:], tri)
            nc.sync.dma_start(iden_sb[:], iden)
            for dc in range(8):
                nc.sync.dma_start(xt_tiles[dc][:, 512:1024],
                                  xT[dc * 128:(dc + 1) * 128, 512:1024])
            nc.sync.dma_start(cos_sb[:, 512:1024], cosf[:, 512:1024])
            nc.sync.dma_start(sin_sb[:, 512:1024], sins[:, 512:1024])
            for dc in range(8):
                nc.sync.dma_start(xt_tiles[dc][:, 1024:S],
                                  xT[dc * 128:(dc + 1) * 128, 1024:S])
            nc.sync.dma_start(cos_sb[:, 1024:S], cosf[:, 1024:S])
            nc.sync.dma_start(sin_sb[:, 1024:S], sins[:, 1024:S])
            nc.sync.dma_start(wo_sb[:], woT.rearrange("(fc p) n -> p fc n", p=128))

            # warm the exp table (input: first-loaded tile, so this runs early)
            warm = cpool.tile([1, 1], F32, tag="warm")
            nc.scalar.activation(warm[:], wqk_sb[0:1, 0, 0:1], AF.Exp, scale=1.0)
            nc.gpsimd.memset(zrow_sb[:], 0.0)

            for _it in range(iters):
                # [64, 8, S]: head-halves in free slots, all at partition base 0
                # (bf16 matmul operands at partition base 64 hang the device)
                qkT = bpool.tile([64, 8, S], BF16, tag="qkT", name=f"qkT{_it}")
                attnT = bpool.tile([128, 2, S], BF16, tag="attnT", name=f"attnT{_it}")
                vt = bpool.tile([128, NKB, GH, 65], BF16, tag="vt", name=f"vt{_it}")
                # ones column for row sums: contiguous whole-tile memset, V
                # copies then overwrite cols 0:64 (strided memset avoided)
                nc.gpsimd.memset(vt[:].rearrange("p a b c -> p (a b c)"), 1.0)

                # A psum accumulators: 2 live banks (qb parity), tail bytes
                # reused as bf16 transpose target
                def a_tile(qb):
                    return psum.tile([128, 512], F32, tag=f"A{qb % 2}", bufs=1,
                                     name=f"A{qb % 2}")

                A_map = {}
                pending_o = []

                def emit_pv(qt, kb, pr):
                    """PV matmuls of kb into the A accumulators of q tile qt."""
                    q0 = qt * QT
                    qb_lo = max(kb, q0 // KB)
                    for pair in range(2):
                        for qb in range(qb_lo, (q0 + QT) // KB):
                            A = A_map[qb]
                            Av = A[:].rearrange("p (h n) -> p h n", h=4)
                            poff = qb * KB - q0
                            for h2 in range(2):
                                h = pair * 2 + h2
                                last = (kb == qb and pair == 1 and h2 == 1)
                                nc.tensor.matmul(
                                    Av[:, h, 0:65],
                                    pr[:, pair, h2, poff:poff + 128],
                                    vt[:, kb, h, :],
                                    start=False, stop=last,
                                    skip_group_check=not last)

                def attention_step(qt, pending):
                    """scores/exp for q tile qt; PV pipelined one kb behind.
                    pending: list of deferred emitters (flushed after kb=0
                    scores to hide their latency)."""
                    q0 = qt * QT
                    nkb = (q0 + QT) // KB  # kb in [0, nkb)
                    inflight = []          # [(kb, prs)] awaiting PV, depth 2
                    for kb in range(nkb):
                        diag = kb * KB >= q0  # kb inside this q tile's diagonal
                        lam = max(kb * KB - q0, 0)
                        ss = psum.tile([128, 2, 2, QT], F32, tag="ss", bufs=2)
                        qsl = slice(q0 + lam, q0 + QT)
                        ksl = slice(kb * KB, (kb + 1) * KB)
                        for pair in range(2):
                            for half in range(2):
                                qs, ks = 2 * pair + half, 4 + 2 * pair + half
                                nc.tensor.matmul(ss[:, pair, half, lam:QT],
                                                 qkT[:, ks, ksl],
                                                 qkT[:, qs, qsl],
                                                 start=True, stop=True)
                        pr = ppool.tile([128, 2, 2, QT], BF16, tag="pr")
                        if lam == 0:
                            # contiguous: single exp over the whole tile
                            nc.scalar.activation(
                                pr[:].rearrange("p a b n -> p (a b n)"),
                                ss[:].rearrange("p a b n -> p (a b n)"),
                                AF.Exp, scale=0.125)
                        else:
                            # diagonal: per-pair exp keeps the AP at 2 free dims
                            for pair in range(2):
                                nc.scalar.activation(pr[:, pair, :, lam:QT],
                                                     ss[:, pair, :, lam:QT],
                                                     AF.Exp, scale=0.125)
                        if diag:
                            dsl = slice(lam, lam + 128)
                            for pair in range(2):
                                nc.vector.tensor_tensor(
                                    pr[:, pair, :, dsl], pr[:, pair, :, dsl],
                                    tri_sb[:, None, :].to_broadcast([128, 2, 128]),
                                    MULT)

                        inflight.append((kb, pr))
                        if len(inflight) > 2:
                            pkb, ppr = inflight.pop(0)
                            emit_pv(qt, pkb, ppr)
                    for pkb, ppr in inflight:
                        emit_pv(qt, pkb, ppr)
                    for qb in range(q0 // KB, (q0 + QT) // KB):
                        pending.append(finalize_qb(qb))

                def finalize_qb(qb):
                    """normalize + transpose into attnT; returns deferred O-proj."""
                    A = A_map.pop(qb)
                    Av = A[:].rearrange("p (h n) -> p h n", h=4)
                    rcp = wpool.tile([128, 4], F32, tag="rcp")
                    nc.vector.reciprocal(rcp[:], Av[:, :, 64])
                    asb = wpool.tile([128, 4, 64], BF16, tag="asb")
                    nc.vector.tensor_tensor(
                        asb[:], Av[:, :, 0:64],
                        rcp[:, :, None].to_broadcast([128, 4, 64]), MULT)
                    qsl = slice(qb * KB, (qb + 1) * KB)
                    for ft in range(2):
                        # transpose lands in the tail bytes of the drained A bank
                        pT = A[:, 384 + 64 * ft:448 + 64 * ft].bitcast(BF16)
                        nc.tensor.matmul(pT, asb[:, 2 * ft:2 * ft + 2, :]
                                         .rearrange("p a b -> p (a b)"),
                                         iden_sb[:], is_transpose=True,
                                         start=True, stop=True,
                                         skip_group_check=True)
                        nc.vector.tensor_copy(attnT[:, ft, qsl], pT)

                    def o_proj():
                        for nh in range(2):
                            nsl = slice(nh * 512, (nh + 1) * 512)
                            pso = psum.tile([128, 512], F32, tag="mm", bufs=2)
                            nc.tensor.matmul(pso[:], attnT[:, 0, qsl], wo_sb[:, 0, nsl],
                                             start=True, stop=False)
                            nc.tensor.matmul(pso[:], attnT[:, 1, qsl], wo_sb[:, 1, nsl],
                                             start=False, stop=True)
                            ob = opool.tile([128, 512], BF16, tag="osb")
                            nc.vector.tensor_copy(ob[:], pso[:])
                            nc.sync.dma_start(out[qsl, nsl], ob[:])
                    return o_proj

                # ---------------- main pipeline over t (s-tiles of 512) -------
                def v_proj(t, sbp):
                    kb0 = 4 * t + 2 * sbp
                    psv = psum.tile([128, 512], F32, tag="mm", bufs=2)
                    pv_v = psv[:].rearrange("p (b n) -> p b n", b=2)
                    for par in range(2):
                        sb = kb0 + par
                        for dc in range(8):
                            nc.tensor.matmul(
                                pv_v[:, par, :],
                                xt_tiles[dc][:, sb * 128:(sb + 1) * 128],
                                wv_sb[:, dc, :], start=(dc == 0), stop=(dc == 7))
                    nc.vector.tensor_copy(
                        vt[:, kb0:kb0 + 2, :, 0:64],
                        pv_v[:].rearrange("p b (h d) -> p b h d", h=4))

                for t in range(4):
                    tsl = slice(t * 512, (t + 1) * 512)
                    # QK projection + rope, V projection interleaved
                    for c in (0, 2, 1, 3):
                        if t == 0 and c < 2:
                            ps = psum.tile([128, 512], F32, tag=f"A{c}", bufs=1,
                                           name=f"psA{c}")
                        else:
                            ps = psum.tile([128, 512], F32, tag="mm", bufs=2)
                        for dc in range(8):
                            nc.tensor.matmul(
                                ps[:], wqk_sb[:, dc, c * 128:(c + 1) * 128],
                                xt_tiles[dc][:, tsl],
                                start=(dc == 0), stop=(dc == 7))
                        sh = rpool.tile([128, 512], F32, tag="sh")
                        nc.vector.stream_shuffle(sh[:], ps[:], SWAP_MASK)
                        m1 = rpool.tile([128, 512], BF16, tag="m1")
                        nc.vector.tensor_tensor(m1[:], ps[:], cos_sb[:, tsl], MULT)
                        m2 = rpool.tile([128, 512], BF16, tag="m2")
                        nc.gpsimd.tensor_tensor(m2[:], sh[:], sin_sb[:, tsl], MULT)
                        # c -> slots: Q pairs at 2c, K pairs at 2c (c>=2 maps +4)
                        nc.gpsimd.tensor_tensor(qkT[:, 2 * c, tsl],
                                                m1[0:64, :], m2[0:64, :], ADD)
                        nc.vector.tensor_tensor(qkT[:, 2 * c + 1, tsl],
                                                m1[64:128, :], m2[64:128, :], ADD)
                        if c in (1, 3):
                            v_proj(t, (c - 1) // 2)
                    # attention for the two q tiles covered by this t
                    for qhalf in range(2):
                        qt = 2 * t + qhalf
                        for qb in range(2 * qt, 2 * qt + 2):
                            A_map[qb] = a_tile(qb)
                            # explicit zeroing matmul: real HW has no
                            # region-level zero-on-start; every PV matmul
                            # accumulates (start=False) onto this
                            nc.tensor.matmul(A_map[qb][:], zrow_sb[0:1, 0:128],
                                             zrow_sb[0:1, :], start=True,
                                             stop=False)
                        attention_step(qt, pending_o)
                for fn in pending_o:
                    fn()
                pending_o.clear()

    nc.compile()
    return nc


def _host_tables(token_positions):
    pos = np.asarray(token_positions, dtype=np.float32)  # [S]
    freq = THETA ** (-np.arange(0, DK, 2, dtype=np.float32) / DK)  # [32]
    f64 = np.repeat(freq, 2)          # [64] freq per feature index
    ang64 = pos[None, :] * f64[:, None]  # [64, S]
    cos64 = np.cos(ang64)
    sin64 = np.sin(ang64)
    # v2 applies the swap BEFORE the sin multiply (shuffle reads the psum
    # directly), so the folded sign pattern is flipped vs the baseline:
    # row 2i needs -sin, row 2i+1 needs +sin.
    sign = np.where(np.arange(DK) % 2 == 0, -1.0, 1.0).astype(np.float32)
    sins64 = sin64 * sign[:, None]
    cosf = np.concatenate([cos64, cos64], axis=0)   # [128, S]
    sins = np.concatenate([sins64, sins64], axis=0)  # [128, S]
    return cosf.astype(NPBF), sins.astype(NPBF)


def kernel(x, Wq, Wk, Wv, Wo, token_positions):
    x = np.asarray(x, dtype=np.float32)
    Wq = np.asarray(Wq, dtype=np.float32)
    Wk = np.asarray(Wk, dtype=np.float32)
    Wv = np.asarray(Wv, dtype=np.float32)
    Wo = np.asarray(Wo, dtype=np.float32)

    if "nc" not in _CACHED:
        _CACHED["nc"] = _build_nc(iters=int(os.environ.get("BENCH_ITERS", "1")))
    nc = _CACHED["nc"]

    cosf, sins = _host_tables(token_positions)
    tri = np.triu(np.ones((128, 128), dtype=np.float32)).astype(NPBF)
    iden = np.eye(128, dtype=NPBF)

    xT = [np.ascontiguousarray(x[b].T).astype(NPBF) for b in range(B)]
    in_maps = []
    for c in range(NCORES):
        b, g = c // GROUPS, c % GROUPS
        R = slice(g * GF, (g + 1) * GF)
        wqkT = np.ascontiguousarray(
            np.concatenate([Wq[R].T, Wk[R].T], axis=1)).astype(NPBF)  # [D, 512]
        wvT = np.ascontiguousarray(Wv[R].T).astype(NPBF)              # [D, 256]
        woT = np.ascontiguousarray(Wo[:, R].T).astype(NPBF)           # [256, D]
        in_maps.append({
            "xT": xT[b], "wqkT": wqkT, "wvT": wvT, "woT": woT,
            "cosf": cosf, "sins": sins, "tri": tri, "iden": iden,
            "cachebust": np.zeros((int(os.environ.get("BENCH_ITERS", "1")), KVER),
                                  dtype=np.float32),
        })

    try:
        res = run_bass_kernel_spmd(nc, in_maps, core_ids=list(range(NCORES)))
    except Exception:
        import time as _time
        _time.sleep(2.0)
        res = run_bass_kernel_spmd(nc, in_maps, core_ids=list(range(NCORES)))
    _CACHED["last_results"] = res
    outs = [np.asarray(r["out"], dtype=np.float32) for r in res.results]
    full = np.empty((B, S, D), dtype=np.float32)
    for b in range(B):
        full[b] = sum(outs[b * GROUPS + g] for g in range(GROUPS))
    return full
